# revision 13
# baseline (speedup 1.0000x reference)
"""CP/PARAFAC bilinear regression kernel for Trainium2 (8 NeuronCores).

Computes y[n] = beta_0 + sum_{i,j} x[n,i,j] * w[i,j],  w = gamma^T @ alpha.

Data-parallel over the batch axis: each of the 8 cores gets 16384 rows of x.

The reduction is HBM-bandwidth-bound; the kernel keeps the DMA engines
streaming and does the arithmetic on the tensor engine (own SBUF ports):

- Host: x is cast to fp16 and laid out feature-planar per core:
  xt[c, k, n] = x_row[n, c*128 + k], features zero-padded 448 -> 512 so each
  chunk c puts 128 features on 128 SBUF partitions, rows contiguous per
  feature. This makes every device DMA a clean >=8KB-per-partition load.
- Device: w = gamma^T @ alpha is computed in fp32 on the PE, cast to fp16.
  Main loop: one 4 MiB DMA per 4096-row group; per group the PE runs
  4 stationary loads (w chunk [128,1]) x 8 moving windows (x^T [128,512])
  = 32 matmuls of N=512, accumulating y windows in PSUM [1,512] (fp32,
  8 banks in flight). A per-window DVE tensor_scalar folds +beta_0 and
  writes the y row [1,16384]; one contiguous DMA stores it.

Accuracy: fp16 quantization of x (~2.3e-4) and w (~2.1e-4) only; all
accumulation is fp32 in PSUM.
"""

import numpy as np

N_TOTAL = 131072
N_CORES = 8
N_PER_CORE = N_TOTAL // N_CORES  # 16384
NG = 7
NA = 64
RANK = 64
D = NG * NA  # 448
DP = 512  # padded feature dim
NCH = DP // 128  # 4 feature chunks
P = 128
GROUP_ROWS = 4096
N_GROUPS = N_PER_CORE // GROUP_ROWS  # 4
WIN = 512  # rows per PSUM window (one bank: 512 f32)
WINS_PER_GROUP = GROUP_ROWS // WIN  # 8

_CACHE = {}


def _build():
    from concourse import bacc, mybir, tile

    f32 = mybir.dt.float32
    f16 = mybir.dt.float16

    nc = bacc.Bacc("TRN2", target_bir_lowering=False, debug=False)

    xt_d = nc.dram_tensor("xt", [NCH, P, N_PER_CORE], f16, kind="ExternalInput").ap()
    gamma_d = nc.dram_tensor("gamma", [RANK, NG], f32, kind="ExternalInput").ap()
    alpha_d = nc.dram_tensor("alpha", [RANK, NA], f32, kind="ExternalInput").ap()
    beta_d = nc.dram_tensor("beta", [1], f32, kind="ExternalInput").ap()
    y_d = nc.dram_tensor("y", [N_PER_CORE], f32, kind="ExternalOutput").ap()
    w_d = nc.dram_tensor("w_scratch", [D], f32).ap()

    add = mybir.AluOpType.add

    with tile.TileContext(nc) as tc:
        with (
            tc.tile_pool(name="const", bufs=1) as cpool,
            tc.tile_pool(name="xp", bufs=2) as xpool,
            tc.tile_pool(name="ps", bufs=8, space="PSUM") as pspool,
        ):
            # ---- w = gamma^T @ alpha on the PE (K = RANK = 64), fp32
            g_sb = cpool.tile([RANK, NG], f32)
            a_sb = cpool.tile([RANK, NA], f32)
            nc.sync.dma_start(out=g_sb[:], in_=gamma_d[:, :])
            nc.sync.dma_start(out=a_sb[:], in_=alpha_d[:, :])
            # stage through DVE so the PE matmul needs only ONE sem wait
            ga_sb = cpool.tile([RANK, NG + NA], f32)
            nc.vector.tensor_copy(out=ga_sb[:, :NG], in_=g_sb[:])
            nc.vector.tensor_copy(out=ga_sb[:, NG:], in_=a_sb[:])
            w_ps = pspool.tile([NG, NA], f32, name="psw", tag="psw")
            nc.tensor.matmul(
                w_ps[:], ga_sb[:, :NG], ga_sb[:, NG:], start=True, stop=True
            )
            w_sb = cpool.tile([NG, NA], f32)
            nc.scalar.copy(out=w_sb[:], in_=w_ps[:])
            nc.sync.dma_start(out=w_d.rearrange("(i j) -> i j", i=NG), in_=w_sb[:])

            # ---- bounce w back as [128 feats, 4 chunks] (flat f = c*128 + k)
            w32 = cpool.tile([P, NCH], f32)
            nc.vector.memset(w32[:], 0.0)
            nc.sync.dma_start(
                out=w32[:, : NCH - 1],
                in_=w_d[: 3 * P].rearrange("(c k) -> k c", c=NCH - 1),
            )
            nc.sync.dma_start(
                out=w32[: D - 3 * P, NCH - 1 :],
                in_=w_d[3 * P :][:, None],
            )
            w16 = cpool.tile([P, NCH], f16)
            nc.vector.tensor_copy(out=w16[:], in_=w32[:])

            beta_sb = cpool.tile([1, 1], f32)
            nc.sync.dma_start(out=beta_sb[:], in_=beta_d[None, :])

            y_row = cpool.tile([1, N_PER_CORE], f32)

            # ---- main loop: one 4 MiB DMA per group; PE streams x as moving
            for g in range(N_GROUPS):
                xt = xpool.tile([P, NCH, GROUP_ROWS], f16)
                nc.sync.dma_start(
                    out=xt[:],
                    in_=xt_d[:, :, g * GROUP_ROWS : (g + 1) * GROUP_ROWS].rearrange(
                        "c k j -> k c j"
                    ),
                )
                psums = [
                    pspool.tile([1, WIN], f32, name="psw", tag="psw") for _ in range(WINS_PER_GROUP)
                ]
                for c in range(NCH):
                    for w in range(WINS_PER_GROUP):
                        nc.tensor.matmul(
                            psums[w][:],
                            w16[:, c : c + 1],
                            xt[:, c, w * WIN : (w + 1) * WIN],
                            start=(c == 0),
                            stop=(c == NCH - 1),
                        )
                for w in range(WINS_PER_GROUP):
                    j0 = (g * WINS_PER_GROUP + w) * WIN
                    nc.vector.tensor_scalar(
                        out=y_row[:, j0 : j0 + WIN],
                        in0=psums[w][:],
                        scalar1=beta_sb[:],
                        scalar2=None,
                        op0=add,
                    )

            nc.sync.dma_start(out=y_d[None, :], in_=y_row[:])

    nc.compile()
    return nc


def _prep_x(x):
    """Full x [131072, 7, 64] f32 -> per-core planar fp16 [4, 128, 16384]:
    xt[c, k, n] = x[core_base + n, flat=c*128+k], zero-padded to 512 feats."""
    xf = np.asarray(x, dtype=np.float32).reshape(N_TOTAL, D)
    out = []
    for i in range(N_CORES):
        a = xf[i * N_PER_CORE : (i + 1) * N_PER_CORE]
        ap = np.zeros((N_PER_CORE, DP), dtype=np.float16)
        ap[:, :D] = a
        out.append(np.ascontiguousarray(ap.T).reshape(NCH, P, N_PER_CORE))
    return out


def _make_in_maps(x, beta_0, gamma, alpha):
    xt_shards = _prep_x(x)
    gamma_np = np.ascontiguousarray(np.asarray(gamma, dtype=np.float32))
    alpha_np = np.ascontiguousarray(np.asarray(alpha, dtype=np.float32))
    beta_np = np.asarray(beta_0, dtype=np.float32).reshape(1)
    return [
        {
            "xt": xt_shards[i],
            "gamma": gamma_np,
            "alpha": alpha_np,
            "beta": beta_np,
        }
        for i in range(N_CORES)
    ]


def kernel(x, beta_0, gamma, alpha):
    from concourse.bass_utils import run_bass_kernel_spmd

    if "nc" not in _CACHE:
        _CACHE["nc"] = _build()
    nc = _CACHE["nc"]

    in_maps = _make_in_maps(x, beta_0, gamma, alpha)
    res = run_bass_kernel_spmd(nc, in_maps, list(range(N_CORES)))
    y = np.concatenate([res.results[i]["y"] for i in range(N_CORES)])
    return y.astype(np.float32)
